# revision 14
# baseline (speedup 1.0000x reference)
"""Trainium2 Bass kernel for nn_FFTConv: y = tanh(Re(ifft(fft(u)*Ks)) + D*u).

Self-contained: builds constant tables with numpy, shards over 8 NeuronCores
(H-parallel: 32 channels/core), runs a Bass/Tile kernel per core via
run_bass_kernel_spmd, gathers the full output.

Algorithm (per core):
  Prologue:
    G[p,m] = 1/(1 - A_p * WL^m)           (P=64 poles x L=8192 freqs, on DVE/ACT)
    Ks[h]  = BC[h,:] @ G                  (TensorE, -> DRAM scratch, complex)
  Main loop processes a PAIR of channels (hA, hB) x 4 batch rows per group,
  stacking the two channels along the partition dim so every elementwise op
  runs on the full 128 partitions.  2-stage matmul FFT, L = 128*64:
    n = n1 + 128*n2 ; m = k2 + 64*k1
    partition layout in stage 1/4: (h, n2) ; in stage 2/3: k1 / o2.
    Y1 = blkdiag(F64) @ u                 [(h k2), (b n1)]
    Y2 = Y1 * T                           twiddle T[k2,n1] = WL^(n1*k2) (x2 stacked)
    per b: full-width PE transpose        [(h k2), n1] -> [n1, (h k2)]
    X  = F128 @ Y2t                       [k1, (b h k2)]
    S  = X * Ks[h].reshape(128,64)        spectral, PSUM read direct
    Z1 = conj(F128) @ S                   [o2, (b h k2)]
    Z2 = Z1 * conj(TI)                    TI[o2,k2] = WL^(k2*o2)
    per b: full-width PE transpose        [o2, (h k2)] -> [(h k2), o2]
    xo = (blkdiag(F64r)/L) @ Z2tr + (blkdiag(F64i)/L) @ Z2ti   (real part, 1/L folded)
    y  = tanh(xo + D[h]*u)
"""
import os
import sys
import numpy as np

for p in ("/opt/trn_rl_repo", "/root/.axon_site/_ro/trn_rl_repo"):
    if os.path.isdir(p) and p not in sys.path:
        sys.path.append(p)

B, H, L, P = 16, 256, 8192, 64
NCORES = 8
HSH = H // NCORES          # 32 channels per core
NPAIR = HSH // 2           # 16 channel pairs per core
GB = 4                     # batch rows per group
NG = B // GB               # 4 groups per pair
F32R = os.environ.get("KERNEL_F32R", "1") == "1"   # reduced-precision fast matmul mode
REPEAT = int(os.environ.get("KERNEL_REPEAT", "1"))  # repeat main loop (timing only)
MIDBUFS = int(os.environ.get("KERNEL_MIDBUFS", "2"))
IOBUFS = int(os.environ.get("KERNEL_IOBUFS", "3"))
TPACC = os.environ.get("KERNEL_TPACC", "1") == "1"  # accumulating transpose pairs

_CACHE = {}


def _tables():
    a64 = np.arange(64)
    a128 = np.arange(128)
    th64 = 2 * np.pi * np.outer(a64, a64) / 64.0
    th128 = 2 * np.pi * np.outer(a128, a128) / 128.0
    thT = 2 * np.pi * np.outer(a64, a128) / L       # [k2, n1]
    thTI = 2 * np.pi * np.outer(a128, a64) / L      # [o2, k2]

    def blk(m):
        z = np.zeros((128, 128))
        z[:64, :64] = m
        z[64:, 64:] = m
        return z

    f64r, f64i = np.cos(th64), -np.sin(th64)
    t = {
        "w64r": blk(f64r), "w64i": blk(f64i),
        "w64rs": blk(f64r) / L, "w64is": blk(f64i) / L,
        "f128r": np.cos(th128), "f128i": -np.sin(th128), "f128in": np.sin(th128),
        "f128rn": -np.cos(th128),
        # fwd twiddle stacked twice along partitions: [(h k2), n1]
        "tr2": np.tile(np.cos(thT), (2, 1)), "ti2": np.tile(-np.sin(thT), (2, 1)),
        "ti2n": np.tile(np.sin(thT), (2, 1)),
        # conj(TI) passed directly: re=cos, im=+sin (and negated-imag variant)
        "tir": np.cos(thTI), "tii": np.sin(thTI), "tiin": -np.sin(thTI),
        "i128": np.eye(128),
    }
    m = np.arange(L)
    cm = np.cos(2 * np.pi * m / L).reshape(2, 4096)
    sm = np.sin(2 * np.pi * m / L).reshape(2, 4096)
    # pre-replicated across 64 partitions per half: (128, 4096)
    t["cm"] = np.repeat(cm, 64, axis=0)
    t["sm"] = np.repeat(sm, 64, axis=0)
    return {k: v.astype(np.float32) for k, v in t.items()}


TBL_SHAPES = {
    "w64r": [128, 128], "w64i": [128, 128],
    "w64rs": [128, 128], "w64is": [128, 128],
    "f128r": [128, 128], "f128i": [128, 128], "f128in": [128, 128],
    "f128rn": [128, 128],
    "tr2": [128, 128], "ti2": [128, 128], "ti2n": [128, 128],
    "tir": [128, 64], "tii": [128, 64], "tiin": [128, 64],
    "i128": [128, 128],
    "cm": [128, 4096], "sm": [128, 4096],
}
MM_TBLS = ("w64r", "w64i", "w64rs", "w64is", "f128r", "f128i", "f128in", "f128rn", "i128")


def _build(nc_mod):
    """Builds the Bass program (same program for all cores)."""
    bass, tile, mybir, bacc = nc_mod
    dt = mybir.dt
    f32 = dt.float32
    MMDT = dt.float32r if F32R else dt.float32

    def mdt(ap):
        return ap.bitcast(MMDT) if F32R else ap

    nc = bacc.Bacc("TRN2", target_bir_lowering=False, debug=False)
    AF = mybir.ActivationFunctionType
    OP = mybir.AluOpType

    # ---------------- DRAM parameters ----------------
    u_d = nc.declare_dram_parameter("u_sh", [B, HSH, L], f32, isOutput=False)
    y_d = nc.declare_dram_parameter("y_sh", [B, HSH, L], f32, isOutput=True)
    ar_d = nc.declare_dram_parameter("a_re", [2 * P, 1], f32, isOutput=False)
    ai_d = nc.declare_dram_parameter("a_im", [2 * P, 1], f32, isOutput=False)
    bcr_d = nc.declare_dram_parameter("bct_r", [P, HSH], f32, isOutput=False)
    bci_d = nc.declare_dram_parameter("bct_i", [P, HSH], f32, isOutput=False)
    bcin_d = nc.declare_dram_parameter("bct_i_neg", [P, HSH], f32, isOutput=False)
    # D arranged per channel pair: [128, NPAIR]; rows 0:64 = D[2hp], 64:128 = D[2hp+1]
    d_d = nc.declare_dram_parameter("d_pair", [128, NPAIR], f32, isOutput=False)
    tbl_d = {n: nc.declare_dram_parameter(n, TBL_SHAPES[n], f32, isOutput=False)
             for n in TBL_SHAPES}

    ksr_d = nc.dram_tensor("ksr_scratch", [HSH, L], f32)
    ksi_d = nc.dram_tensor("ksi_scratch", [HSH, L], f32)

    with tile.TileContext(nc) as tc:
        with (
            tc.tile_pool(name="const", bufs=1) as cpool,
            tc.tile_pool(name="ks", bufs=4) as ksp,
        ):
            # ---------- load constants ----------
            tb = {}
            for n in TBL_SHAPES:
                if n in ("cm", "sm"):
                    continue
                tb[n] = cpool.tile(TBL_SHAPES[n], f32, tag=f"c_{n}", name=f"c_{n}")
                if n in MM_TBLS:
                    nc.sync.dma_start(mdt(tb[n][:]), mdt(tbl_d[n][:]))
                else:
                    nc.sync.dma_start(tb[n][:], tbl_d[n][:])
            a_re = cpool.tile([128, 1], f32, tag="a_re")
            a_im = cpool.tile([128, 1], f32, tag="a_im")
            nc.sync.dma_start(a_re[:], ar_d[:])
            nc.sync.dma_start(a_im[:], ai_d[:])
            bct_r = cpool.tile([P, HSH], f32, tag="bct_r")
            bct_i = cpool.tile([P, HSH], f32, tag="bct_i")
            bct_in = cpool.tile([P, HSH], f32, tag="bct_in")
            nc.sync.dma_start(mdt(bct_r[:]), mdt(bcr_d[:]))
            nc.sync.dma_start(mdt(bct_i[:]), mdt(bci_d[:]))
            nc.sync.dma_start(mdt(bct_in[:]), mdt(bcin_d[:]))
            d_pr = cpool.tile([128, NPAIR], f32, tag="d_pr")
            nc.sync.dma_start(d_pr[:], d_d[:])

            prologue_pools = (
                tc.tile_pool(name="gwork", bufs=1),
                tc.tile_pool(name="psk", bufs=2, space=bass.MemorySpace.PSUM),
            )
            gpool = prologue_pools[0].__enter__()
            pskp = prologue_pools[1].__enter__()
            # 1 + |A|^2 per partition (stacked twice)
            one_a2 = cpool.tile([128, 1], f32, tag="one_a2")
            t_sq = gpool.tile([128, 1], f32, tag="g_sq")
            nc.scalar.activation(one_a2[:], a_re[:], AF.Square)
            nc.scalar.activation(t_sq[:], a_im[:], AF.Square)
            nc.vector.tensor_tensor(one_a2[:], one_a2[:], t_sq[:], OP.add)
            nc.vector.tensor_scalar_add(one_a2[:], one_a2[:], 1.0)

            # ---------- G = 1/(1 - A*WL^m), layout (128p=[p|p], 4096f) ----------
            ctab = gpool.tile([128, 4096], f32, tag="g_ctab")
            stab = gpool.tile([128, 4096], f32, tag="g_stab")
            nc.sync.dma_start(ctab[:], tbl_d["cm"][:])
            nc.sync.dma_start(stab[:], tbl_d["sm"][:])
            gq = gpool.tile([128, 4096], f32, tag="g_q")
            gt = gpool.tile([128, 4096], f32, tag="g_t")
            gdr = gpool.tile([128, 4096], f32, tag="g_dr")
            gn2 = gpool.tile([128, 4096], f32, tag="g_n2")
            g_r = gpool.tile([128, 4096], f32, tag="g_r")
            g_i = gpool.tile([128, 4096], f32, tag="g_i")
            # q = A_re*C + A_im*S
            nc.vector.tensor_scalar_mul(gq[:], ctab[:], a_re[:])
            nc.scalar.activation(gt[:], stab[:], AF.Identity, scale=a_im[:])
            nc.vector.tensor_tensor(gq[:], gq[:], gt[:], OP.add)
            # dr = 1 - q ; n2 = 1+|A|^2 - 2q ; rn = 1/n2
            nc.scalar.activation(gdr[:], gq[:], AF.Identity, scale=-1.0, bias=1.0)
            nc.vector.tensor_scalar(gn2[:], gq[:], -2.0, one_a2[:], OP.mult, OP.add)
            nc.vector.reciprocal(gn2[:], gn2[:])
            # di_n = A_im*C - A_re*S  (numerator of +Gi)
            g_t4 = gpool.tile([128, 4096], f32, tag="g_t4")
            nc.scalar.activation(gt[:], ctab[:], AF.Identity, scale=a_im[:])
            nc.scalar.activation(g_t4[:], stab[:], AF.Identity, scale=a_re[:])
            nc.vector.tensor_tensor(gt[:], gt[:], g_t4[:], OP.subtract)
            nc.vector.tensor_tensor(mdt(g_r[:]), gdr[:], gn2[:], OP.mult)
            nc.vector.tensor_tensor(mdt(g_i[:]), gt[:], gn2[:], OP.mult)

            # hi halves to base-partition-0 tiles (matmul rhs must match lhsT base)
            g_r_hi = gpool.tile([64, 4096], f32, tag="g_r_hi")
            g_i_hi = gpool.tile([64, 4096], f32, tag="g_i_hi")
            nc.sync.dma_start(mdt(g_r_hi[:]), mdt(g_r[64:128, :]))
            nc.sync.dma_start(mdt(g_i_hi[:]), mdt(g_i[64:128, :]))

            # ---------- Ks rows = BC @ G -> DRAM scratch ----------
            for j in range(16):  # m chunks of 512
                half = j // 8
                foff = (j % 8) * 512
                gr_sl = (g_r if half == 0 else g_r_hi)[0:64, foff:foff + 512]
                gi_sl = (g_i if half == 0 else g_i_hi)[0:64, foff:foff + 512]
                kr = pskp.tile([HSH, 512], f32, tag="ks_ps")
                ki = pskp.tile([HSH, 512], f32, tag="ks_ps")
                nc.tensor.matmul(kr[:], mdt(bct_r[:]), mdt(gr_sl), start=True, stop=False)
                nc.tensor.matmul(kr[:], mdt(bct_in[:]), mdt(gi_sl), start=False, stop=True)
                nc.tensor.matmul(ki[:], mdt(bct_i[:]), mdt(gr_sl), start=True, stop=False)
                nc.tensor.matmul(ki[:], mdt(bct_r[:]), mdt(gi_sl), start=False, stop=True)
                krs = ksp.tile([HSH, 512], f32, tag="ks_sb")
                kis = ksp.tile([HSH, 512], f32, tag="ks_sb")
                nc.scalar.activation(krs[:], kr[:], AF.Copy)
                nc.scalar.activation(kis[:], ki[:], AF.Copy)
                nc.sync.dma_start(ksr_d[:, j * 512:(j + 1) * 512], krs[:])
                nc.sync.dma_start(ksi_d[:, j * 512:(j + 1) * 512], kis[:])

            prologue_pools[1].__exit__(None, None, None)
            prologue_pools[0].__exit__(None, None, None)
            main_pools = (
                tc.tile_pool(name="io", bufs=IOBUFS),
                tc.tile_pool(name="mid", bufs=MIDBUFS),
                tc.tile_pool(name="up", bufs=5),
                tc.tile_pool(name="ps", bufs=2, space=bass.MemorySpace.PSUM),
            )
            iop = main_pools[0].__enter__()
            midp = main_pools[1].__enter__()
            upp = main_pools[2].__enter__()
            psp = main_pools[3].__enter__()

            i128m = mdt(tb["i128"][:])
            tr_b = tb["tr2"][:].unsqueeze(1).broadcast_to([128, GB, 128])
            ti_b = tb["ti2"][:].unsqueeze(1).broadcast_to([128, GB, 128])
            tin_b = tb["ti2n"][:].unsqueeze(1).broadcast_to([128, GB, 128])
            tir_b = tb["tir"][:].unsqueeze(1).unsqueeze(1).broadcast_to([128, GB, 2, 64])
            tii_b = tb["tii"][:].unsqueeze(1).unsqueeze(1).broadcast_to([128, GB, 2, 64])
            tiin_b = tb["tiin"][:].unsqueeze(1).unsqueeze(1).broadcast_to([128, GB, 2, 64])

            # ---------- software-pipelined main loop (skew 3) ----------
            # S0: u load, fwd stage 1, fwd twiddle products
            # S1: fwd transposes(+combine), y2t copies, stage 2, spectral products
            # S2: inv stage 1 (+spectral combine), inv twiddle products
            # S3: inv transposes(+combine), z2t copies, inv stage 2, final, store

            def s0(ctx):
                hp, g = ctx["hp"], ctx["g"]
                hA, hB = 2 * hp, 2 * hp + 1
                bsl = slice(g * GB, (g + 1) * GB)
                u_t = upp.tile([128, GB, 128], f32, tag="u_t", name="u_t")
                nc.sync.dma_start(
                    mdt(u_t[0:64]),
                    mdt(u_d[bsl, hA, :].rearrange("b (n2 n1) -> n2 b n1", n1=128)))
                nc.sync.dma_start(
                    mdt(u_t[64:128]),
                    mdt(u_d[bsl, hB, :].rearrange("b (n2 n1) -> n2 b n1", n1=128)))
                u_flat = u_t[:].rearrange("p b f -> p (b f)")

                y1r = psp.tile([128, GB, 128], f32, tag="ps_y1", name="y1r")
                y1i = psp.tile([128, GB, 128], f32, tag="ps_y1", name="y1i")
                nc.tensor.matmul(y1r[:].rearrange("p b f -> p (b f)"),
                                 mdt(tb["w64r"][:]), mdt(u_flat))
                nc.tensor.matmul(y1i[:].rearrange("p b f -> p (b f)"),
                                 mdt(tb["w64i"][:]), mdt(u_flat))

                # y2r = y1r*tr2 - y1i*ti2 = ta + tb2 ; y2i = y1r*ti2 + y1i*tr2 = tc + td
                c_y1i = midp.tile([128, GB, 128], f32, tag="cp_a", name="c_y1i")
                nc.scalar.activation(c_y1i[:].rearrange("p b f -> p (b f)"),
                                     y1i[:].rearrange("p b f -> p (b f)"), AF.Copy)
                ta = midp.tile([128, GB, 128], f32, tag="ftw_a", name="ta")
                tb2 = midp.tile([128, GB, 128], f32, tag="ftw_b", name="tb2")
                tc_ = midp.tile([128, GB, 128], f32, tag="ftw_c", name="tc_")
                td = midp.tile([128, GB, 128], f32, tag="ftw_d", name="td")
                nc.vector.tensor_tensor(mdt(ta[:]), y1r[:], tr_b, OP.mult)
                nc.vector.tensor_tensor(mdt(tc_[:]), y1r[:], ti_b, OP.mult)
                nc.gpsimd.tensor_tensor(mdt(tb2[:]), c_y1i[:], tin_b, OP.mult)
                nc.gpsimd.tensor_tensor(mdt(td[:]), c_y1i[:], tr_b, OP.mult)
                ctx.update(u_flat=u_flat, ta=ta, tb2=tb2, tc_=tc_, td=td)

            def s1(ctx):
                ta, tb2, tc_, td = ctx["ta"], ctx["tb2"], ctx["tc_"], ctx["td"]
                ksr_b, ksi_b = ctx["kb"]
                y2t_ps_r = psp.tile([128, GB, 128], f32, tag="ps_mid", name="y2t_ps_r")
                y2t_ps_i = psp.tile([128, GB, 128], f32, tag="ps_mid", name="y2t_ps_i")
                if TPACC:
                    for j2 in range(GB):
                        nc.tensor.matmul(mdt(y2t_ps_r[:, j2, :]), mdt(ta[:, j2, :]), i128m,
                                         is_transpose=True, start=True, stop=False)
                        nc.tensor.matmul(mdt(y2t_ps_r[:, j2, :]), mdt(tb2[:, j2, :]), i128m,
                                         is_transpose=True, start=False, stop=True)
                        nc.tensor.matmul(mdt(y2t_ps_i[:, j2, :]), mdt(tc_[:, j2, :]), i128m,
                                         is_transpose=True, start=True, stop=False)
                        nc.tensor.matmul(mdt(y2t_ps_i[:, j2, :]), mdt(td[:, j2, :]), i128m,
                                         is_transpose=True, start=False, stop=True)
                else:
                    y2r = midp.tile([128, GB, 128], f32, tag="y2r", name="y2r")
                    y2i = midp.tile([128, GB, 128], f32, tag="y2i", name="y2i")
                    nc.gpsimd.tensor_tensor(mdt(y2r[:]), ta[:], tb2[:], OP.add)
                    nc.vector.tensor_tensor(mdt(y2i[:]), tc_[:], td[:], OP.add)
                    for j2 in range(GB):
                        nc.tensor.matmul(mdt(y2t_ps_r[:, j2, :]), mdt(y2r[:, j2, :]), i128m,
                                         is_transpose=True)
                        nc.tensor.matmul(mdt(y2t_ps_i[:, j2, :]), mdt(y2i[:, j2, :]), i128m,
                                         is_transpose=True)
                y2tr = iop.tile([128, GB, 128], f32, tag="y2tr", name="y2tr")
                y2ti = iop.tile([128, GB, 128], f32, tag="y2ti", name="y2ti")
                nc.scalar.activation(mdt(y2tr[:].rearrange("p b f -> p (b f)")),
                                     y2t_ps_r[:].rearrange("p b f -> p (b f)"), AF.Copy)
                nc.scalar.activation(mdt(y2ti[:].rearrange("p b f -> p (b f)")),
                                     y2t_ps_i[:].rearrange("p b f -> p (b f)"), AF.Copy)
                y2tr_f = y2tr[:].rearrange("p b f -> p (b f)")
                y2ti_f = y2ti[:].rearrange("p b f -> p (b f)")

                xr = psp.tile([128, GB, 2, 64], f32, tag="ps_x", name="xr")
                xi = psp.tile([128, GB, 2, 64], f32, tag="ps_x", name="xi")
                xr_f = xr[:].rearrange("p b h k -> p (b h k)")
                xi_f = xi[:].rearrange("p b h k -> p (b h k)")
                nc.tensor.matmul(xr_f, mdt(tb["f128r"][:]), mdt(y2tr_f), start=True, stop=False)
                nc.tensor.matmul(xr_f, mdt(tb["f128in"][:]), mdt(y2ti_f), start=False, stop=True)
                nc.tensor.matmul(xi_f, mdt(tb["f128i"][:]), mdt(y2tr_f), start=True, stop=False)
                nc.tensor.matmul(xi_f, mdt(tb["f128r"][:]), mdt(y2ti_f), start=False, stop=True)

                # spectral products: sr = sa - sb ; si = sc2 + sd (combined in inv1)
                c_xi = midp.tile([128, GB, 2, 64], f32, tag="cp_b", name="c_xi")
                nc.scalar.activation(c_xi[:].rearrange("p b h k -> p (b h k)"),
                                     xi_f, AF.Copy)
                sa = midp.tile([128, GB, 2, 64], f32, tag="sp_a", name="sa")
                sb = midp.tile([128, GB, 2, 64], f32, tag="sp_b", name="sb")
                sc2 = midp.tile([128, GB, 2, 64], f32, tag="sp_c", name="sc2")
                sd = midp.tile([128, GB, 2, 64], f32, tag="sp_d", name="sd")
                nc.vector.tensor_tensor(mdt(sa[:]), xr[:], ksr_b, OP.mult)
                nc.vector.tensor_tensor(mdt(sc2[:]), xr[:], ksi_b, OP.mult)
                nc.gpsimd.tensor_tensor(mdt(sb[:]), c_xi[:], ksi_b, OP.mult)
                nc.gpsimd.tensor_tensor(mdt(sd[:]), c_xi[:], ksr_b, OP.mult)
                ctx.update(sa=sa, sb=sb, sc2=sc2, sd=sd)

            def s2(ctx):
                sa_f = ctx["sa"][:].rearrange("p b h k -> p (b h k)")
                sb_f = ctx["sb"][:].rearrange("p b h k -> p (b h k)")
                sc_f = ctx["sc2"][:].rearrange("p b h k -> p (b h k)")
                sd_f = ctx["sd"][:].rearrange("p b h k -> p (b h k)")
                # z1r = f128r@sa + f128rn@sb + f128i@sc2 + f128i@sd
                # z1i = f128in@sa + f128i@sb + f128r@sc2 + f128r@sd
                z1r = psp.tile([128, GB, 2, 64], f32, tag="ps_mid", name="z1r")
                z1i = psp.tile([128, GB, 2, 64], f32, tag="ps_mid", name="z1i")
                z1r_f = z1r[:].rearrange("p b h k -> p (b h k)")
                z1i_f = z1i[:].rearrange("p b h k -> p (b h k)")
                nc.tensor.matmul(z1r_f, mdt(tb["f128r"][:]), mdt(sa_f), start=True, stop=False)
                nc.tensor.matmul(z1r_f, mdt(tb["f128rn"][:]), mdt(sb_f), start=False, stop=False)
                nc.tensor.matmul(z1r_f, mdt(tb["f128i"][:]), mdt(sc_f), start=False, stop=False)
                nc.tensor.matmul(z1r_f, mdt(tb["f128i"][:]), mdt(sd_f), start=False, stop=True)
                nc.tensor.matmul(z1i_f, mdt(tb["f128in"][:]), mdt(sa_f), start=True, stop=False)
                nc.tensor.matmul(z1i_f, mdt(tb["f128i"][:]), mdt(sb_f), start=False, stop=False)
                nc.tensor.matmul(z1i_f, mdt(tb["f128r"][:]), mdt(sc_f), start=False, stop=False)
                nc.tensor.matmul(z1i_f, mdt(tb["f128r"][:]), mdt(sd_f), start=False, stop=True)

                # z2r = z1r*tir - z1i*tii = za + zb2 ; z2i = z1r*tii + z1i*tir = zc + zd
                c_z1i = midp.tile([128, GB, 2, 64], f32, tag="cp_c", name="c_z1i")
                nc.scalar.activation(c_z1i[:].rearrange("p b h k -> p (b h k)"),
                                     z1i_f, AF.Copy)
                za = midp.tile([128, GB, 2, 64], f32, tag="itw_a", name="za")
                zb2 = midp.tile([128, GB, 2, 64], f32, tag="itw_b", name="zb2")
                zc = midp.tile([128, GB, 2, 64], f32, tag="itw_c", name="zc")
                zd = midp.tile([128, GB, 2, 64], f32, tag="itw_d", name="zd")
                nc.vector.tensor_tensor(mdt(za[:]), z1r[:], tir_b, OP.mult)
                nc.vector.tensor_tensor(mdt(zc[:]), z1r[:], tii_b, OP.mult)
                nc.gpsimd.tensor_tensor(mdt(zb2[:]), c_z1i[:], tiin_b, OP.mult)
                nc.vector.tensor_tensor(mdt(zd[:]), c_z1i[:], tir_b, OP.mult)
                ctx.update(za=za, zb2=zb2, zc=zc, zd=zd)

            def s3(ctx):
                hp, g = ctx["hp"], ctx["g"]
                hA, hB = 2 * hp, 2 * hp + 1
                bsl = slice(g * GB, (g + 1) * GB)
                za, zb2, zc, zd = ctx["za"], ctx["zb2"], ctx["zc"], ctx["zd"]
                z2t_ps_r = psp.tile([128, GB, 128], f32, tag="ps_out", name="z2t_ps_r")
                z2t_ps_i = psp.tile([128, GB, 128], f32, tag="ps_out", name="z2t_ps_i")
                if TPACC:
                    for j2 in range(GB):
                        nc.tensor.matmul(mdt(z2t_ps_r[:, j2, :]),
                                         mdt(za[:, j2, :, :].rearrange("p h k -> p (h k)")),
                                         i128m, is_transpose=True, start=True, stop=False)
                        nc.tensor.matmul(mdt(z2t_ps_r[:, j2, :]),
                                         mdt(zb2[:, j2, :, :].rearrange("p h k -> p (h k)")),
                                         i128m, is_transpose=True, start=False, stop=True)
                        nc.tensor.matmul(mdt(z2t_ps_i[:, j2, :]),
                                         mdt(zc[:, j2, :, :].rearrange("p h k -> p (h k)")),
                                         i128m, is_transpose=True, start=True, stop=False)
                        nc.tensor.matmul(mdt(z2t_ps_i[:, j2, :]),
                                         mdt(zd[:, j2, :, :].rearrange("p h k -> p (h k)")),
                                         i128m, is_transpose=True, start=False, stop=True)
                else:
                    z2r = midp.tile([128, GB, 2, 64], f32, tag="z2r", name="z2r")
                    z2i = midp.tile([128, GB, 2, 64], f32, tag="z2i", name="z2i")
                    nc.gpsimd.tensor_tensor(mdt(z2r[:]), za[:], zb2[:], OP.add)
                    nc.vector.tensor_tensor(mdt(z2i[:]), zc[:], zd[:], OP.add)
                    for j2 in range(GB):
                        nc.tensor.matmul(mdt(z2t_ps_r[:, j2, :]),
                                         mdt(z2r[:, j2, :, :].rearrange("p h k -> p (h k)")),
                                         i128m, is_transpose=True)
                        nc.tensor.matmul(mdt(z2t_ps_i[:, j2, :]),
                                         mdt(z2i[:, j2, :, :].rearrange("p h k -> p (h k)")),
                                         i128m, is_transpose=True)
                z2tr = iop.tile([128, GB, 128], f32, tag="z2tr", name="z2tr")
                z2ti = iop.tile([128, GB, 128], f32, tag="z2ti", name="z2ti")
                nc.scalar.activation(mdt(z2tr[:].rearrange("p b f -> p (b f)")),
                                     z2t_ps_r[:].rearrange("p b f -> p (b f)"), AF.Copy)
                nc.scalar.activation(mdt(z2ti[:].rearrange("p b f -> p (b f)")),
                                     z2t_ps_i[:].rearrange("p b f -> p (b f)"), AF.Copy)

                xo_ps = psp.tile([128, GB, 128], f32, tag="ps_out", name="xo_ps")
                xo_f = xo_ps[:].rearrange("p b f -> p (b f)")
                nc.tensor.matmul(xo_f, mdt(tb["w64rs"][:]),
                                 mdt(z2tr[:].rearrange("p b f -> p (b f)")),
                                 start=True, stop=False)
                nc.tensor.matmul(xo_f, mdt(tb["w64is"][:]),
                                 mdt(z2ti[:].rearrange("p b f -> p (b f)")),
                                 start=False, stop=True)

                yt = midp.tile([128, GB, 128], f32, tag="yt", name="yt")
                nc.vector.scalar_tensor_tensor(yt[:].rearrange("p b f -> p (b f)"),
                                               ctx["u_flat"], d_pr[:, hp:hp + 1], xo_f,
                                               OP.mult, OP.add)
                yo = iop.tile([128, GB, 128], f32, tag="yo", name="yo")
                nc.scalar.activation(yo[:].rearrange("p b f -> p (b f)"),
                                     yt[:].rearrange("p b f -> p (b f)"), AF.Tanh)
                nc.sync.dma_start(
                    y_d[bsl, hA, :].rearrange("b (n2 n1) -> n2 b n1", n1=128), yo[0:64])
                nc.sync.dma_start(
                    y_d[bsl, hB, :].rearrange("b (n2 n1) -> n2 b n1", n1=128), yo[64:128])

            def prep_pair(hp):
                hA = 2 * hp
                # Ks for the pair: [k1, (h k2)] = [128, 2, 64]
                ksr_t = ksp.tile([128, 2, 64], f32, tag="ks_h", name="ksr_t")
                ksi_t = ksp.tile([128, 2, 64], f32, tag="ks_h", name="ksi_t")
                nc.sync.dma_start(
                    ksr_t[:], ksr_d[hA:hA + 2, :].rearrange("h (k1 k2) -> k1 h k2", k2=64))
                nc.sync.dma_start(
                    ksi_t[:], ksi_d[hA:hA + 2, :].rearrange("h (k1 k2) -> k1 h k2", k2=64))
                return (
                    ksr_t[:].unsqueeze(1).broadcast_to([128, GB, 2, 64]),
                    ksi_t[:].unsqueeze(1).broadcast_to([128, GB, 2, 64]),
                )

            TOT = NPAIR * NG
            for _rep in range(REPEAT):
                ctxs = {}
                kb = None
                for t in range(TOT + 3):
                    if t < TOT:
                        hp, g = divmod(t, NG)
                        if g == 0:
                            kb = prep_pair(hp)
                        ctxs[t] = {"hp": hp, "g": g, "kb": kb}
                        s0(ctxs[t])
                    if 0 <= t - 1 < TOT:
                        s1(ctxs[t - 1])
                    if 0 <= t - 2 < TOT:
                        s2(ctxs[t - 2])
                    if 0 <= t - 3 < TOT:
                        s3(ctxs[t - 3])
                        del ctxs[t - 3]
            for mp in reversed(main_pools):
                mp.__exit__(None, None, None)

    nc.compile()
    return nc


def _get_program():
    key = ("prog", F32R, REPEAT, MIDBUFS, IOBUFS)
    if key not in _CACHE:
        import concourse.bass as bass
        import concourse.tile as tile
        from concourse import mybir, bacc
        _CACHE[key] = _build((bass, tile, mybir, bacc))
    return _CACHE[key]


def _make_d_pair(D_sh):
    """[128, NPAIR]: rows 0:64 = D[2hp], rows 64:128 = D[2hp+1]."""
    d = np.zeros((128, NPAIR), dtype=np.float32)
    for hp in range(NPAIR):
        d[0:64, hp] = D_sh[2 * hp]
        d[64:128, hp] = D_sh[2 * hp + 1]
    return d


def kernel(u, A_re, A_im, BC_re, BC_im, D):
    from concourse.bass_utils import run_bass_kernel_spmd

    u = np.ascontiguousarray(u, dtype=np.float32)
    tabs = _tables()
    nc = _get_program()

    in_maps = []
    for c in range(NCORES):
        hs = slice(c * HSH, (c + 1) * HSH)
        m = {
            "u_sh": np.ascontiguousarray(u[:, hs, :]),
            "a_re": np.ascontiguousarray(
                np.concatenate([A_re, A_re]).reshape(2 * P, 1).astype(np.float32)),
            "a_im": np.ascontiguousarray(
                np.concatenate([A_im, A_im]).reshape(2 * P, 1).astype(np.float32)),
            "bct_r": np.ascontiguousarray(BC_re[hs].T.astype(np.float32)),
            "bct_i": np.ascontiguousarray(BC_im[hs].T.astype(np.float32)),
            "bct_i_neg": np.ascontiguousarray(-BC_im[hs].T.astype(np.float32)),
            "d_pair": _make_d_pair(np.asarray(D[hs], dtype=np.float32)),
        }
        m.update(tabs)
        in_maps.append(m)

    res = None
    last_err = None
    for attempt in range(3):
        try:
            res = run_bass_kernel_spmd(nc, in_maps, list(range(NCORES)))
            break
        except Exception as e:  # transient NRT_EXEC_UNIT_UNRECOVERABLE flakes
            last_err = e
            import time as _time
            _time.sleep(2.0)
    if res is None:
        raise last_err
    out = np.concatenate([res.results[c]["y_sh"] for c in range(NCORES)], axis=1)
    return out.astype(np.float32)


if __name__ == "__main__":
    rng = np.random.default_rng(0)
    u = rng.standard_normal((B, H, L), dtype=np.float32)
    A_re = rng.uniform(0.5, 0.99, P).astype(np.float32)
    A_im = rng.uniform(-0.5, 0.5, P).astype(np.float32)
    BC_re = rng.standard_normal((H, P), dtype=np.float32)
    BC_im = rng.standard_normal((H, P), dtype=np.float32)
    D = rng.uniform(0, 1, H).astype(np.float32)
    y = kernel(u=u, A_re=A_re, A_im=A_im, BC_re=BC_re, BC_im=BC_im, D=D)
    print("out", y.shape, y.dtype)


# revision 15
# speedup vs baseline: 14.5379x; 14.5379x over previous
"""Trainium2 Bass kernel for nn_FFTConv: y = tanh(Re(ifft(fft(u)*Ks)) + D*u).

Self-contained: builds constant tables with numpy, shards over 8 NeuronCores
(H-parallel: 32 channels/core), runs a Bass/Tile kernel per core via
run_bass_kernel_spmd, gathers the full output.

Algorithm (per core):
  Prologue:
    G[p,m] = 1/(1 - A_p * WL^m)           (P=64 poles x L=8192 freqs, on DVE/ACT)
    Ks[h]  = BC[h,:] @ G                  (TensorE, -> DRAM scratch, complex)
  Main loop processes a PAIR of channels (hA, hB) x 4 batch rows per group,
  stacking the two channels along the partition dim so every elementwise op
  runs on the full 128 partitions.  2-stage matmul FFT, L = 128*64:
    n = n1 + 128*n2 ; m = k2 + 64*k1
    partition layout in stage 1/4: (h, n2) ; in stage 2/3: k1 / o2.
    Y1 = blkdiag(F64) @ u                 [(h k2), (b n1)]
    Y2 = Y1 * T                           twiddle T[k2,n1] = WL^(n1*k2) (x2 stacked)
    per b: full-width PE transpose        [(h k2), n1] -> [n1, (h k2)]
    X  = F128 @ Y2t                       [k1, (b h k2)]
    S  = X * Ks[h].reshape(128,64)        spectral, PSUM read direct
    Z1 = conj(F128) @ S                   [o2, (b h k2)]
    Z2 = Z1 * conj(TI)                    TI[o2,k2] = WL^(k2*o2)
    per b: full-width PE transpose        [o2, (h k2)] -> [(h k2), o2]
    xo = (blkdiag(F64r)/L) @ Z2tr + (blkdiag(F64i)/L) @ Z2ti   (real part, 1/L folded)
    y  = tanh(xo + D[h]*u)
"""
import os
import sys
import numpy as np

for p in ("/opt/trn_rl_repo", "/root/.axon_site/_ro/trn_rl_repo"):
    if os.path.isdir(p) and p not in sys.path:
        sys.path.append(p)

B, H, L, P = 16, 256, 8192, 64
NCORES = 8
HSH = H // NCORES          # 32 channels per core
NPAIR = HSH // 2           # 16 channel pairs per core
GB = 4                     # batch rows per group
NG = B // GB               # 4 groups per pair
F32R = os.environ.get("KERNEL_F32R", "1") == "1"   # reduced-precision fast matmul mode
REPEAT = int(os.environ.get("KERNEL_REPEAT", "1"))  # repeat main loop (timing only)
MIDBUFS = int(os.environ.get("KERNEL_MIDBUFS", "2"))
IOBUFS = int(os.environ.get("KERNEL_IOBUFS", "3"))
TPACC = os.environ.get("KERNEL_TPACC", "1") == "1"  # accumulating transpose pairs

_CACHE = {}


def _tables():
    a64 = np.arange(64)
    a128 = np.arange(128)
    th64 = 2 * np.pi * np.outer(a64, a64) / 64.0
    th128 = 2 * np.pi * np.outer(a128, a128) / 128.0
    thT = 2 * np.pi * np.outer(a64, a128) / L       # [k2, n1]
    thTI = 2 * np.pi * np.outer(a128, a64) / L      # [o2, k2]

    def blk(m):
        z = np.zeros((128, 128))
        z[:64, :64] = m
        z[64:, 64:] = m
        return z

    f64r, f64i = np.cos(th64), -np.sin(th64)
    t = {
        "w64r": blk(f64r), "w64i": blk(f64i),
        "w64rs": blk(f64r) / L, "w64is": blk(f64i) / L,
        "f128r": np.cos(th128), "f128i": -np.sin(th128), "f128in": np.sin(th128),
        "f128rn": -np.cos(th128),
        # fwd twiddle stacked twice along partitions: [(h k2), n1]
        "tr2": np.tile(np.cos(thT), (2, 1)), "ti2": np.tile(-np.sin(thT), (2, 1)),
        "ti2n": np.tile(np.sin(thT), (2, 1)),
        # conj(TI) passed directly: re=cos, im=+sin (and negated-imag variant)
        "tir": np.tile(np.cos(thTI), (1, 2)), "tii": np.tile(np.sin(thTI), (1, 2)),
        "tiin": np.tile(-np.sin(thTI), (1, 2)),
        "i128": np.eye(128),
    }
    m = np.arange(L)
    cm = np.cos(2 * np.pi * m / L).reshape(2, 4096)
    sm = np.sin(2 * np.pi * m / L).reshape(2, 4096)
    # pre-replicated across 64 partitions per half: (128, 4096)
    t["cm"] = np.repeat(cm, 64, axis=0)
    t["sm"] = np.repeat(sm, 64, axis=0)
    return {k: v.astype(np.float32) for k, v in t.items()}


TBL_SHAPES = {
    "w64r": [128, 128], "w64i": [128, 128],
    "w64rs": [128, 128], "w64is": [128, 128],
    "f128r": [128, 128], "f128i": [128, 128], "f128in": [128, 128],
    "f128rn": [128, 128],
    "tr2": [128, 128], "ti2": [128, 128], "ti2n": [128, 128],
    "tir": [128, 128], "tii": [128, 128], "tiin": [128, 128],
    "i128": [128, 128],
    "cm": [128, 4096], "sm": [128, 4096],
}
MM_TBLS = ("w64r", "w64i", "w64rs", "w64is", "f128r", "f128i", "f128in", "f128rn", "i128")


def _build(nc_mod):
    """Builds the Bass program (same program for all cores)."""
    bass, tile, mybir, bacc = nc_mod
    dt = mybir.dt
    f32 = dt.float32
    MMDT = dt.float32r if F32R else dt.float32

    def mdt(ap):
        return ap.bitcast(MMDT) if F32R else ap

    nc = bacc.Bacc("TRN2", target_bir_lowering=False, debug=False)
    AF = mybir.ActivationFunctionType
    OP = mybir.AluOpType

    # ---------------- DRAM parameters ----------------
    u_d = nc.declare_dram_parameter("u_sh", [B, HSH, L], f32, isOutput=False)
    y_d = nc.declare_dram_parameter("y_sh", [B, HSH, L], f32, isOutput=True)
    ar_d = nc.declare_dram_parameter("a_re", [2 * P, 1], f32, isOutput=False)
    ai_d = nc.declare_dram_parameter("a_im", [2 * P, 1], f32, isOutput=False)
    bcr_d = nc.declare_dram_parameter("bct_r", [P, HSH], f32, isOutput=False)
    bci_d = nc.declare_dram_parameter("bct_i", [P, HSH], f32, isOutput=False)
    bcin_d = nc.declare_dram_parameter("bct_i_neg", [P, HSH], f32, isOutput=False)
    # D arranged per channel pair: [128, NPAIR]; rows 0:64 = D[2hp], 64:128 = D[2hp+1]
    d_d = nc.declare_dram_parameter("d_pair", [128, NPAIR], f32, isOutput=False)
    tbl_d = {n: nc.declare_dram_parameter(n, TBL_SHAPES[n], f32, isOutput=False)
             for n in TBL_SHAPES}

    ksr_d = nc.dram_tensor("ksr_scratch", [HSH, L], f32)
    ksi_d = nc.dram_tensor("ksi_scratch", [HSH, L], f32)

    with tile.TileContext(nc) as tc:
        with (
            tc.tile_pool(name="const", bufs=1) as cpool,
            tc.tile_pool(name="ks", bufs=4) as ksp,
        ):
            # ---------- load constants ----------
            tb = {}
            for n in TBL_SHAPES:
                if n in ("cm", "sm"):
                    continue
                tb[n] = cpool.tile(TBL_SHAPES[n], f32, tag=f"c_{n}", name=f"c_{n}")
                if n in MM_TBLS:
                    nc.sync.dma_start(mdt(tb[n][:]), mdt(tbl_d[n][:]))
                else:
                    nc.sync.dma_start(tb[n][:], tbl_d[n][:])
            a_re = cpool.tile([128, 1], f32, tag="a_re")
            a_im = cpool.tile([128, 1], f32, tag="a_im")
            nc.sync.dma_start(a_re[:], ar_d[:])
            nc.sync.dma_start(a_im[:], ai_d[:])
            bct_r = cpool.tile([P, HSH], f32, tag="bct_r")
            bct_i = cpool.tile([P, HSH], f32, tag="bct_i")
            bct_in = cpool.tile([P, HSH], f32, tag="bct_in")
            nc.sync.dma_start(mdt(bct_r[:]), mdt(bcr_d[:]))
            nc.sync.dma_start(mdt(bct_i[:]), mdt(bci_d[:]))
            nc.sync.dma_start(mdt(bct_in[:]), mdt(bcin_d[:]))
            d_pr = cpool.tile([128, NPAIR], f32, tag="d_pr")
            nc.sync.dma_start(d_pr[:], d_d[:])

            prologue_pools = (
                tc.tile_pool(name="gwork", bufs=1),
                tc.tile_pool(name="psk", bufs=2, space=bass.MemorySpace.PSUM),
            )
            gpool = prologue_pools[0].__enter__()
            pskp = prologue_pools[1].__enter__()
            # 1 + |A|^2 per partition (stacked twice)
            one_a2 = cpool.tile([128, 1], f32, tag="one_a2")
            t_sq = gpool.tile([128, 1], f32, tag="g_sq")
            nc.scalar.activation(one_a2[:], a_re[:], AF.Square)
            nc.scalar.activation(t_sq[:], a_im[:], AF.Square)
            nc.vector.tensor_tensor(one_a2[:], one_a2[:], t_sq[:], OP.add)
            nc.vector.tensor_scalar_add(one_a2[:], one_a2[:], 1.0)

            # ---------- G = 1/(1 - A*WL^m), layout (128p=[p|p], 4096f) ----------
            ctab = gpool.tile([128, 4096], f32, tag="g_ctab")
            stab = gpool.tile([128, 4096], f32, tag="g_stab")
            nc.sync.dma_start(ctab[:], tbl_d["cm"][:])
            nc.sync.dma_start(stab[:], tbl_d["sm"][:])
            gq = gpool.tile([128, 4096], f32, tag="g_q")
            gt = gpool.tile([128, 4096], f32, tag="g_t")
            gdr = gpool.tile([128, 4096], f32, tag="g_dr")
            gn2 = gpool.tile([128, 4096], f32, tag="g_n2")
            g_r = gpool.tile([128, 4096], f32, tag="g_r")
            g_i = gpool.tile([128, 4096], f32, tag="g_i")
            # q = A_re*C + A_im*S
            nc.vector.tensor_scalar_mul(gq[:], ctab[:], a_re[:])
            nc.scalar.activation(gt[:], stab[:], AF.Identity, scale=a_im[:])
            nc.vector.tensor_tensor(gq[:], gq[:], gt[:], OP.add)
            # dr = 1 - q ; n2 = 1+|A|^2 - 2q ; rn = 1/n2
            nc.scalar.activation(gdr[:], gq[:], AF.Identity, scale=-1.0, bias=1.0)
            nc.vector.tensor_scalar(gn2[:], gq[:], -2.0, one_a2[:], OP.mult, OP.add)
            nc.vector.reciprocal(gn2[:], gn2[:])
            # di_n = A_im*C - A_re*S  (numerator of +Gi)
            g_t4 = gpool.tile([128, 4096], f32, tag="g_t4")
            nc.scalar.activation(gt[:], ctab[:], AF.Identity, scale=a_im[:])
            nc.scalar.activation(g_t4[:], stab[:], AF.Identity, scale=a_re[:])
            nc.vector.tensor_tensor(gt[:], gt[:], g_t4[:], OP.subtract)
            nc.vector.tensor_tensor(mdt(g_r[:]), gdr[:], gn2[:], OP.mult)
            nc.vector.tensor_tensor(mdt(g_i[:]), gt[:], gn2[:], OP.mult)

            # hi halves to base-partition-0 tiles (matmul rhs must match lhsT base)
            g_r_hi = gpool.tile([64, 4096], f32, tag="g_r_hi")
            g_i_hi = gpool.tile([64, 4096], f32, tag="g_i_hi")
            nc.sync.dma_start(mdt(g_r_hi[:]), mdt(g_r[64:128, :]))
            nc.sync.dma_start(mdt(g_i_hi[:]), mdt(g_i[64:128, :]))

            # ---------- Ks rows = BC @ G -> DRAM scratch ----------
            for j in range(16):  # m chunks of 512
                half = j // 8
                foff = (j % 8) * 512
                gr_sl = (g_r if half == 0 else g_r_hi)[0:64, foff:foff + 512]
                gi_sl = (g_i if half == 0 else g_i_hi)[0:64, foff:foff + 512]
                kr = pskp.tile([HSH, 512], f32, tag="ks_ps")
                ki = pskp.tile([HSH, 512], f32, tag="ks_ps")
                nc.tensor.matmul(kr[:], mdt(bct_r[:]), mdt(gr_sl), start=True, stop=False)
                nc.tensor.matmul(kr[:], mdt(bct_in[:]), mdt(gi_sl), start=False, stop=True)
                nc.tensor.matmul(ki[:], mdt(bct_i[:]), mdt(gr_sl), start=True, stop=False)
                nc.tensor.matmul(ki[:], mdt(bct_r[:]), mdt(gi_sl), start=False, stop=True)
                krs = ksp.tile([HSH, 512], f32, tag="ks_sb")
                kis = ksp.tile([HSH, 512], f32, tag="ks_sb")
                nc.scalar.activation(krs[:], kr[:], AF.Copy)
                nc.scalar.activation(kis[:], ki[:], AF.Copy)
                nc.sync.dma_start(ksr_d[:, j * 512:(j + 1) * 512], krs[:])
                nc.sync.dma_start(ksi_d[:, j * 512:(j + 1) * 512], kis[:])

            prologue_pools[1].__exit__(None, None, None)
            prologue_pools[0].__exit__(None, None, None)
            main_pools = (
                tc.tile_pool(name="io", bufs=IOBUFS),
                tc.tile_pool(name="mid", bufs=MIDBUFS),
                tc.tile_pool(name="up", bufs=5),
                tc.tile_pool(name="ps", bufs=2, space=bass.MemorySpace.PSUM),
            )
            iop = main_pools[0].__enter__()
            midp = main_pools[1].__enter__()
            upp = main_pools[2].__enter__()
            psp = main_pools[3].__enter__()

            i128m = mdt(tb["i128"][:])
            tr_b = tb["tr2"][:].unsqueeze(1).broadcast_to([128, GB, 128])
            ti_b = tb["ti2"][:].unsqueeze(1).broadcast_to([128, GB, 128])
            tin_b = tb["ti2n"][:].unsqueeze(1).broadcast_to([128, GB, 128])
            tir_b = tb["tir"][:].rearrange("p (h k) -> p h k", k=64)\
                .unsqueeze(1).broadcast_to([128, GB, 2, 64])
            tii_b = tb["tii"][:].rearrange("p (h k) -> p h k", k=64)\
                .unsqueeze(1).broadcast_to([128, GB, 2, 64])
            tiin_b = tb["tiin"][:].rearrange("p (h k) -> p h k", k=64)\
                .unsqueeze(1).broadcast_to([128, GB, 2, 64])

            # ---------- software-pipelined main loop (skew 3) ----------
            # S0: u load, fwd stage 1, fwd twiddle products
            # S1: fwd transposes(+combine), y2t copies, stage 2, spectral products
            # S2: inv stage 1 (+spectral combine), inv twiddle products
            # S3: inv transposes(+combine), z2t copies, inv stage 2, final, store

            def s0(ctx):
                hp, g = ctx["hp"], ctx["g"]
                hA, hB = 2 * hp, 2 * hp + 1
                bsl = slice(g * GB, (g + 1) * GB)
                u_t = upp.tile([128, GB, 128], f32, tag="u_t", name="u_t")
                nc.sync.dma_start(
                    mdt(u_t[0:64]),
                    mdt(u_d[bsl, hA, :].rearrange("b (n2 n1) -> n2 b n1", n1=128)))
                nc.sync.dma_start(
                    mdt(u_t[64:128]),
                    mdt(u_d[bsl, hB, :].rearrange("b (n2 n1) -> n2 b n1", n1=128)))
                u_flat = u_t[:].rearrange("p b f -> p (b f)")

                y1r = psp.tile([128, GB, 128], f32, tag="ps_y1", name="y1r")
                y1i = psp.tile([128, GB, 128], f32, tag="ps_y1", name="y1i")
                nc.tensor.matmul(y1r[:].rearrange("p b f -> p (b f)"),
                                 mdt(tb["w64r"][:]), mdt(u_flat))
                nc.tensor.matmul(y1i[:].rearrange("p b f -> p (b f)"),
                                 mdt(tb["w64i"][:]), mdt(u_flat))

                # y2r = y1r*tr2 - y1i*ti2 = ta + tb2 ; y2i = y1r*ti2 + y1i*tr2 = tc + td
                c_y1i = midp.tile([128, GB, 128], f32, tag="cp_a", name="c_y1i")
                nc.scalar.activation(c_y1i[:].rearrange("p b f -> p (b f)"),
                                     y1i[:].rearrange("p b f -> p (b f)"), AF.Copy)
                c_y1r = midp.tile([128, GB, 128], f32, tag="cp_a2", name="c_y1r")
                nc.scalar.activation(c_y1r[:].rearrange("p b f -> p (b f)"),
                                     y1r[:].rearrange("p b f -> p (b f)"), AF.Copy)
                ta = midp.tile([128, GB, 128], f32, tag="ftw_a", name="ta")
                tb2 = midp.tile([128, GB, 128], f32, tag="ftw_b", name="tb2")
                tc_ = midp.tile([128, GB, 128], f32, tag="ftw_c", name="tc_")
                td = midp.tile([128, GB, 128], f32, tag="ftw_d", name="td")
                nc.vector.tensor_tensor(mdt(ta[:]), c_y1r[:], tr_b, OP.mult)
                nc.vector.tensor_tensor(mdt(tc_[:]), c_y1r[:], ti_b, OP.mult)
                nc.gpsimd.tensor_tensor(mdt(tb2[:]), c_y1i[:], tin_b, OP.mult)
                nc.gpsimd.tensor_tensor(mdt(td[:]), c_y1i[:], tr_b, OP.mult)
                ctx.update(u_flat=u_flat, ta=ta, tb2=tb2, tc_=tc_, td=td)

            def s1(ctx):
                ta, tb2, tc_, td = ctx["ta"], ctx["tb2"], ctx["tc_"], ctx["td"]
                ksr_b, ksi_b = ctx["kb"]
                y2t_ps_r = psp.tile([128, GB, 128], f32, tag="ps_mid", name="y2t_ps_r")
                y2t_ps_i = psp.tile([128, GB, 128], f32, tag="ps_mid", name="y2t_ps_i")
                if TPACC:
                    for j2 in range(GB):
                        nc.tensor.matmul(mdt(y2t_ps_r[:, j2, :]), mdt(ta[:, j2, :]), i128m,
                                         is_transpose=True, start=True, stop=False)
                        nc.tensor.matmul(mdt(y2t_ps_r[:, j2, :]), mdt(tb2[:, j2, :]), i128m,
                                         is_transpose=True, start=False, stop=True)
                        nc.tensor.matmul(mdt(y2t_ps_i[:, j2, :]), mdt(tc_[:, j2, :]), i128m,
                                         is_transpose=True, start=True, stop=False)
                        nc.tensor.matmul(mdt(y2t_ps_i[:, j2, :]), mdt(td[:, j2, :]), i128m,
                                         is_transpose=True, start=False, stop=True)
                else:
                    y2r = midp.tile([128, GB, 128], f32, tag="y2r", name="y2r")
                    y2i = midp.tile([128, GB, 128], f32, tag="y2i", name="y2i")
                    nc.gpsimd.tensor_tensor(mdt(y2r[:]), ta[:], tb2[:], OP.add)
                    nc.vector.tensor_tensor(mdt(y2i[:]), tc_[:], td[:], OP.add)
                    for j2 in range(GB):
                        nc.tensor.matmul(mdt(y2t_ps_r[:, j2, :]), mdt(y2r[:, j2, :]), i128m,
                                         is_transpose=True)
                        nc.tensor.matmul(mdt(y2t_ps_i[:, j2, :]), mdt(y2i[:, j2, :]), i128m,
                                         is_transpose=True)
                y2tr = iop.tile([128, GB, 128], f32, tag="y2tr", name="y2tr")
                y2ti = iop.tile([128, GB, 128], f32, tag="y2ti", name="y2ti")
                nc.scalar.activation(mdt(y2tr[:].rearrange("p b f -> p (b f)")),
                                     y2t_ps_r[:].rearrange("p b f -> p (b f)"), AF.Copy)
                nc.scalar.activation(mdt(y2ti[:].rearrange("p b f -> p (b f)")),
                                     y2t_ps_i[:].rearrange("p b f -> p (b f)"), AF.Copy)
                y2tr_f = y2tr[:].rearrange("p b f -> p (b f)")
                y2ti_f = y2ti[:].rearrange("p b f -> p (b f)")

                xr = psp.tile([128, GB, 2, 64], f32, tag="ps_x", name="xr")
                xi = psp.tile([128, GB, 2, 64], f32, tag="ps_x", name="xi")
                xr_f = xr[:].rearrange("p b h k -> p (b h k)")
                xi_f = xi[:].rearrange("p b h k -> p (b h k)")
                nc.tensor.matmul(xr_f, mdt(tb["f128r"][:]), mdt(y2tr_f), start=True, stop=False)
                nc.tensor.matmul(xr_f, mdt(tb["f128in"][:]), mdt(y2ti_f), start=False, stop=True)
                nc.tensor.matmul(xi_f, mdt(tb["f128i"][:]), mdt(y2tr_f), start=True, stop=False)
                nc.tensor.matmul(xi_f, mdt(tb["f128r"][:]), mdt(y2ti_f), start=False, stop=True)

                # spectral products: sr = sa - sb ; si = sc2 + sd (combined in inv1)
                c_xi = midp.tile([128, GB, 2, 64], f32, tag="cp_b", name="c_xi")
                nc.scalar.activation(c_xi[:].rearrange("p b h k -> p (b h k)"),
                                     xi_f, AF.Copy)
                c_xr = midp.tile([128, GB, 2, 64], f32, tag="cp_b2", name="c_xr")
                nc.scalar.activation(c_xr[:].rearrange("p b h k -> p (b h k)"),
                                     xr_f, AF.Copy)
                sa = midp.tile([128, GB, 2, 64], f32, tag="sp_a", name="sa")
                sb = midp.tile([128, GB, 2, 64], f32, tag="sp_b", name="sb")
                sc2 = midp.tile([128, GB, 2, 64], f32, tag="sp_c", name="sc2")
                sd = midp.tile([128, GB, 2, 64], f32, tag="sp_d", name="sd")
                nc.vector.tensor_tensor(mdt(sa[:]), c_xr[:], ksr_b, OP.mult)
                nc.vector.tensor_tensor(mdt(sc2[:]), c_xr[:], ksi_b, OP.mult)
                nc.gpsimd.tensor_tensor(mdt(sb[:]), c_xi[:], ksi_b, OP.mult)
                nc.gpsimd.tensor_tensor(mdt(sd[:]), c_xi[:], ksr_b, OP.mult)
                ctx.update(sa=sa, sb=sb, sc2=sc2, sd=sd)

            def s2(ctx):
                sa_f = ctx["sa"][:].rearrange("p b h k -> p (b h k)")
                sb_f = ctx["sb"][:].rearrange("p b h k -> p (b h k)")
                sc_f = ctx["sc2"][:].rearrange("p b h k -> p (b h k)")
                sd_f = ctx["sd"][:].rearrange("p b h k -> p (b h k)")
                # z1r = f128r@sa + f128rn@sb + f128i@sc2 + f128i@sd
                # z1i = f128in@sa + f128i@sb + f128r@sc2 + f128r@sd
                z1r = psp.tile([128, GB, 2, 64], f32, tag="ps_mid", name="z1r")
                z1i = psp.tile([128, GB, 2, 64], f32, tag="ps_mid", name="z1i")
                z1r_f = z1r[:].rearrange("p b h k -> p (b h k)")
                z1i_f = z1i[:].rearrange("p b h k -> p (b h k)")
                nc.tensor.matmul(z1r_f, mdt(tb["f128r"][:]), mdt(sa_f), start=True, stop=False)
                nc.tensor.matmul(z1r_f, mdt(tb["f128rn"][:]), mdt(sb_f), start=False, stop=False)
                nc.tensor.matmul(z1r_f, mdt(tb["f128i"][:]), mdt(sc_f), start=False, stop=False)
                nc.tensor.matmul(z1r_f, mdt(tb["f128i"][:]), mdt(sd_f), start=False, stop=True)
                nc.tensor.matmul(z1i_f, mdt(tb["f128in"][:]), mdt(sa_f), start=True, stop=False)
                nc.tensor.matmul(z1i_f, mdt(tb["f128i"][:]), mdt(sb_f), start=False, stop=False)
                nc.tensor.matmul(z1i_f, mdt(tb["f128r"][:]), mdt(sc_f), start=False, stop=False)
                nc.tensor.matmul(z1i_f, mdt(tb["f128r"][:]), mdt(sd_f), start=False, stop=True)

                # z2r = z1r*tir - z1i*tii = za + zb2 ; z2i = z1r*tii + z1i*tir = zc + zd
                c_z1i = midp.tile([128, GB, 2, 64], f32, tag="cp_c", name="c_z1i")
                nc.scalar.activation(c_z1i[:].rearrange("p b h k -> p (b h k)"),
                                     z1i_f, AF.Copy)
                c_z1r = midp.tile([128, GB, 2, 64], f32, tag="cp_c2", name="c_z1r")
                nc.scalar.activation(c_z1r[:].rearrange("p b h k -> p (b h k)"),
                                     z1r_f, AF.Copy)
                za = midp.tile([128, GB, 2, 64], f32, tag="itw_a", name="za")
                zb2 = midp.tile([128, GB, 2, 64], f32, tag="itw_b", name="zb2")
                zc = midp.tile([128, GB, 2, 64], f32, tag="itw_c", name="zc")
                zd = midp.tile([128, GB, 2, 64], f32, tag="itw_d", name="zd")
                nc.vector.tensor_tensor(mdt(za[:]), c_z1r[:], tir_b, OP.mult)
                nc.vector.tensor_tensor(mdt(zc[:]), c_z1r[:], tii_b, OP.mult)
                nc.gpsimd.tensor_tensor(mdt(zb2[:]), c_z1i[:], tiin_b, OP.mult)
                nc.vector.tensor_tensor(mdt(zd[:]), c_z1i[:], tir_b, OP.mult)
                ctx.update(za=za, zb2=zb2, zc=zc, zd=zd)

            def s3(ctx):
                hp, g = ctx["hp"], ctx["g"]
                hA, hB = 2 * hp, 2 * hp + 1
                bsl = slice(g * GB, (g + 1) * GB)
                za, zb2, zc, zd = ctx["za"], ctx["zb2"], ctx["zc"], ctx["zd"]
                z2t_ps_r = psp.tile([128, GB, 128], f32, tag="ps_out", name="z2t_ps_r")
                z2t_ps_i = psp.tile([128, GB, 128], f32, tag="ps_out", name="z2t_ps_i")
                if TPACC:
                    for j2 in range(GB):
                        nc.tensor.matmul(mdt(z2t_ps_r[:, j2, :]),
                                         mdt(za[:, j2, :, :].rearrange("p h k -> p (h k)")),
                                         i128m, is_transpose=True, start=True, stop=False)
                        nc.tensor.matmul(mdt(z2t_ps_r[:, j2, :]),
                                         mdt(zb2[:, j2, :, :].rearrange("p h k -> p (h k)")),
                                         i128m, is_transpose=True, start=False, stop=True)
                        nc.tensor.matmul(mdt(z2t_ps_i[:, j2, :]),
                                         mdt(zc[:, j2, :, :].rearrange("p h k -> p (h k)")),
                                         i128m, is_transpose=True, start=True, stop=False)
                        nc.tensor.matmul(mdt(z2t_ps_i[:, j2, :]),
                                         mdt(zd[:, j2, :, :].rearrange("p h k -> p (h k)")),
                                         i128m, is_transpose=True, start=False, stop=True)
                else:
                    z2r = midp.tile([128, GB, 2, 64], f32, tag="z2r", name="z2r")
                    z2i = midp.tile([128, GB, 2, 64], f32, tag="z2i", name="z2i")
                    nc.gpsimd.tensor_tensor(mdt(z2r[:]), za[:], zb2[:], OP.add)
                    nc.vector.tensor_tensor(mdt(z2i[:]), zc[:], zd[:], OP.add)
                    for j2 in range(GB):
                        nc.tensor.matmul(mdt(z2t_ps_r[:, j2, :]),
                                         mdt(z2r[:, j2, :, :].rearrange("p h k -> p (h k)")),
                                         i128m, is_transpose=True)
                        nc.tensor.matmul(mdt(z2t_ps_i[:, j2, :]),
                                         mdt(z2i[:, j2, :, :].rearrange("p h k -> p (h k)")),
                                         i128m, is_transpose=True)
                z2tr = iop.tile([128, GB, 128], f32, tag="z2tr", name="z2tr")
                z2ti = iop.tile([128, GB, 128], f32, tag="z2ti", name="z2ti")
                nc.scalar.activation(mdt(z2tr[:].rearrange("p b f -> p (b f)")),
                                     z2t_ps_r[:].rearrange("p b f -> p (b f)"), AF.Copy)
                nc.scalar.activation(mdt(z2ti[:].rearrange("p b f -> p (b f)")),
                                     z2t_ps_i[:].rearrange("p b f -> p (b f)"), AF.Copy)

                xo_ps = psp.tile([128, GB, 128], f32, tag="ps_out", name="xo_ps")
                xo_f = xo_ps[:].rearrange("p b f -> p (b f)")
                nc.tensor.matmul(xo_f, mdt(tb["w64rs"][:]),
                                 mdt(z2tr[:].rearrange("p b f -> p (b f)")),
                                 start=True, stop=False)
                nc.tensor.matmul(xo_f, mdt(tb["w64is"][:]),
                                 mdt(z2ti[:].rearrange("p b f -> p (b f)")),
                                 start=False, stop=True)

                yt = midp.tile([128, GB, 128], f32, tag="yt", name="yt")
                nc.vector.scalar_tensor_tensor(yt[:].rearrange("p b f -> p (b f)"),
                                               ctx["u_flat"], d_pr[:, hp:hp + 1], xo_f,
                                               OP.mult, OP.add)
                yo = iop.tile([128, GB, 128], f32, tag="yo", name="yo")
                nc.scalar.activation(yo[:].rearrange("p b f -> p (b f)"),
                                     yt[:].rearrange("p b f -> p (b f)"), AF.Tanh)
                nc.sync.dma_start(
                    y_d[bsl, hA, :].rearrange("b (n2 n1) -> n2 b n1", n1=128), yo[0:64])
                nc.sync.dma_start(
                    y_d[bsl, hB, :].rearrange("b (n2 n1) -> n2 b n1", n1=128), yo[64:128])

            def prep_pair(hp):
                hA = 2 * hp
                # Ks for the pair: [k1, (h k2)] = [128, 2, 64]
                ksr_t = ksp.tile([128, 2, 64], f32, tag="ks_h", name="ksr_t")
                ksi_t = ksp.tile([128, 2, 64], f32, tag="ks_h", name="ksi_t")
                nc.sync.dma_start(
                    ksr_t[:], ksr_d[hA:hA + 2, :].rearrange("h (k1 k2) -> k1 h k2", k2=64))
                nc.sync.dma_start(
                    ksi_t[:], ksi_d[hA:hA + 2, :].rearrange("h (k1 k2) -> k1 h k2", k2=64))
                return (
                    ksr_t[:].unsqueeze(1).broadcast_to([128, GB, 2, 64]),
                    ksi_t[:].unsqueeze(1).broadcast_to([128, GB, 2, 64]),
                )

            TOT = NPAIR * NG
            for _rep in range(REPEAT):
                ctxs = {}
                kb = None
                for t in range(TOT + 3):
                    if t < TOT:
                        hp, g = divmod(t, NG)
                        if g == 0:
                            kb = prep_pair(hp)
                        ctxs[t] = {"hp": hp, "g": g, "kb": kb}
                        s0(ctxs[t])
                    if 0 <= t - 1 < TOT:
                        s1(ctxs[t - 1])
                    if 0 <= t - 2 < TOT:
                        s2(ctxs[t - 2])
                    if 0 <= t - 3 < TOT:
                        s3(ctxs[t - 3])
                        del ctxs[t - 3]
            for mp in reversed(main_pools):
                mp.__exit__(None, None, None)

    nc.compile()
    return nc


def _get_program():
    key = ("prog", F32R, REPEAT, MIDBUFS, IOBUFS)
    if key not in _CACHE:
        import concourse.bass as bass
        import concourse.tile as tile
        from concourse import mybir, bacc
        _CACHE[key] = _build((bass, tile, mybir, bacc))
    return _CACHE[key]


def _make_d_pair(D_sh):
    """[128, NPAIR]: rows 0:64 = D[2hp], rows 64:128 = D[2hp+1]."""
    d = np.zeros((128, NPAIR), dtype=np.float32)
    for hp in range(NPAIR):
        d[0:64, hp] = D_sh[2 * hp]
        d[64:128, hp] = D_sh[2 * hp + 1]
    return d


def kernel(u, A_re, A_im, BC_re, BC_im, D):
    from concourse.bass_utils import run_bass_kernel_spmd

    u = np.ascontiguousarray(u, dtype=np.float32)
    tabs = _tables()
    nc = _get_program()

    in_maps = []
    for c in range(NCORES):
        hs = slice(c * HSH, (c + 1) * HSH)
        m = {
            "u_sh": np.ascontiguousarray(u[:, hs, :]),
            "a_re": np.ascontiguousarray(
                np.concatenate([A_re, A_re]).reshape(2 * P, 1).astype(np.float32)),
            "a_im": np.ascontiguousarray(
                np.concatenate([A_im, A_im]).reshape(2 * P, 1).astype(np.float32)),
            "bct_r": np.ascontiguousarray(BC_re[hs].T.astype(np.float32)),
            "bct_i": np.ascontiguousarray(BC_im[hs].T.astype(np.float32)),
            "bct_i_neg": np.ascontiguousarray(-BC_im[hs].T.astype(np.float32)),
            "d_pair": _make_d_pair(np.asarray(D[hs], dtype=np.float32)),
        }
        m.update(tabs)
        in_maps.append(m)

    res = None
    last_err = None
    for attempt in range(3):
        try:
            res = run_bass_kernel_spmd(nc, in_maps, list(range(NCORES)))
            break
        except Exception as e:  # transient NRT_EXEC_UNIT_UNRECOVERABLE flakes
            last_err = e
            import time as _time
            _time.sleep(2.0)
    if res is None:
        raise last_err
    out = np.concatenate([res.results[c]["y_sh"] for c in range(NCORES)], axis=1)
    return out.astype(np.float32)


if __name__ == "__main__":
    rng = np.random.default_rng(0)
    u = rng.standard_normal((B, H, L), dtype=np.float32)
    A_re = rng.uniform(0.5, 0.99, P).astype(np.float32)
    A_im = rng.uniform(-0.5, 0.5, P).astype(np.float32)
    BC_re = rng.standard_normal((H, P), dtype=np.float32)
    BC_im = rng.standard_normal((H, P), dtype=np.float32)
    D = rng.uniform(0, 1, H).astype(np.float32)
    y = kernel(u=u, A_re=A_re, A_im=A_im, BC_re=BC_re, BC_im=BC_im, D=D)
    print("out", y.shape, y.dtype)
